# revision 9
# baseline (speedup 1.0000x reference)
import sys
if '/opt/trn_rl_repo' not in sys.path:
    sys.path.insert(0, '/opt/trn_rl_repo')
"""Bass/Tile kernel for one transformer block, uniform SPMD program,
software-pipelined across the two query t-tiles.

Each core receives row-permuted inputs (own TOWN rows first) plus
data-driven causal masks/biases, so all 8 cores run the identical NEFF.

Macro-pipeline (emission order == approximate execution order):
  seg1: slabs A,B (LN1+h^T, K, V, Q-tt0)            [PE dense]
  LN1 for slabs C,D (keeps all ACT Sqrts adjacent)
  seg2: attention(tt0)  x  K/V/Q projections of C,D  [exp hides under PE]
  seg3: attention(tt1)  x  out-proj/LN2/FFN1 of tt0
  seg4: FFN2(tt0) x out-proj/LN2(tt1); FFN1(tt1); FFN2(tt1)

LayerNorm gammas/betas are folded into the following projection weights
on the host; weights ship pre-tiled bf16.  Softmax normalization is
deferred: outputs are written unnormalized, per-head denominators are
gathered into one [16, TT] tile, reciprocal'd once, and broadcast back
over each head's 64 feature rows with one small matmul per pair.
Fully-masked / fully-passing far blocks use a per-core additive bias
inside the exp instead of elementwise masks.
"""
from contextlib import ExitStack

import concourse.bass as bass
import concourse.mybir as mybir
import concourse.tile as tile
from concourse.masks import make_identity

F32 = mybir.dt.float32
BF16 = mybir.dt.bfloat16
AF = mybir.ActivationFunctionType
ALU = mybir.AluOpType


class Cfg:
    def __init__(self, T2=2048, C=1024, H=16, F=None, eps=1e-5):
        self.T2, self.C, self.H = T2, C, H
        self.F = 4 * C if F is None else F
        self.HS = 64
        self.eps = eps
        self.TBLK = T2 // 4          # row block (ownL/ownH/otherL/otherH)
        self.TOWN = 2 * self.TBLK    # rows this core owns
        self.TT = self.TBLK          # t-tile width == block
        self.NTT = 2
        self.CB = C // 128
        self.NPAIR = H // 2
        self.NSB = T2 // 128         # s-blocks
        self.SBB = self.TBLK // 128  # s-blocks per row-block
        self.NTB = self.TOWN // 128  # own token-blocks
        self.TTB = self.TT // 128    # token-blocks per t-tile
        self.FC = self.F // 128
        self.DH = min(H, 8)          # heads per V-proj chunk
        self.NCH = max(C // 512, 1)  # c_out chunks (FFN2 + out-proj)
        self.CHW = min(C, 512)
        self.scale = C ** -0.5

    def att_steps(self):
        """Per t-tile: list of (sb, kind, idx).  kind: 'm' = elementwise
        mask msk[idx], 'b' = additive exp-bias mb[:, idx], None = pass."""
        s0 = ([(sb, 'm', sb) for sb in range(4)]            # ownL diag
              + [(8 + j, 'b', j) for j in range(4)])        # othL all/none
        s1 = ([(sb, None, 0) for sb in range(4)]            # ownL pass
              + [(sb, 'm', sb) for sb in range(4, 8)]       # ownH diag
              + [(8 + j, None, 0) for j in range(4)]        # othL pass
              + [(12 + j, 'b', 4 + j) for j in range(4)])   # othH all/none
        return [s0, s1]


def broadcast_ap(ap, parts=128):
    """[N] vector AP -> [parts, N] partition-broadcast AP (DMA source)."""
    return bass.AP(tensor=ap.tensor, offset=ap.offset,
                   ap=[[0, parts]] + list(ap.ap))


def drive(*streams):
    """Interleave generators: streams = (gen, weight) pairs; each round
    advances gen by `weight` yields until all are exhausted."""
    live = [[g, w] for g, w in streams]
    while live:
        for gw in list(live):
            g, w = gw
            try:
                for _ in range(w):
                    next(g)
            except StopIteration:
                live.remove(gw)


def run(g):
    for _ in g:
        pass


def build(nc, cfg: Cfg):
    c = cfg
    steps_tt = c.att_steps()

    def din(name, shape, dt=F32):
        return nc.dram_tensor(name, shape, dt, kind="ExternalInput").ap()

    xp = din("xp", [c.T2, c.C])
    xpb = din("xpb", [c.T2, c.C], BF16)
    masks = din("masks", [128, 8, c.TT], BF16)
    mbias = din("mbias", [128, 8])
    sel = din("sel", [16, c.NPAIR, 128], BF16)
    wq = din("wq", [c.NPAIR, 128, c.CB, 2, 64], BF16)
    wk = din("wk", [c.NPAIR, 128, c.CB, 2, 64], BF16)
    wv = din("wv", [128, c.CB, c.H, 64], BF16)
    bq = din("bq", [128, c.NPAIR])
    bk = din("bk", [128, c.NPAIR])
    bv = din("bv", [c.C], BF16)
    wp = din("wp", [c.NCH, 128, c.CB, c.CHW], BF16)
    bp = din("bp", [c.C])
    w1 = din("w1", [c.FC, 128, c.CB, 128], BF16)
    b1 = din("b1", [128, c.FC])
    w2 = din("w2", [128, c.FC, c.C], BF16)
    b2 = din("b2", [c.C])
    y = nc.dram_tensor("y", [c.TOWN, c.C], BF16, kind="ExternalOutput").ap()

    # slabs: (perm-row base, first s-block, q t-tile | None)
    SLABS = [(0, 0, 0), (1024, 8, None), (512, 4, 1), (1536, 12, None)]

    with tile.TileContext(nc) as tc:
      with ExitStack() as top:
        # LEFT:  consts | msk | x2 | attnT | h2T | [hT+QT0, p1, wstr
        #        (seg1-2)] -> [rT, consts3, wpp, p4, w1s (seg3-4)]
        # RIGHT: qkv(KT/V/QT1) | attw (seg2-3) -> w2p (seg4)
        consts = top.enter_context(tc.tile_pool(name="consts", bufs=1))
        mskp = top.enter_context(tc.tile_pool(name="mskp", bufs=1))
        x2p = top.enter_context(tc.tile_pool(name="x2p", bufs=1))
        attp = top.enter_context(tc.tile_pool(name="attp", bufs=1))
        h2p = top.enter_context(tc.tile_pool(name="h2p", bufs=1))
        es_qkv = ExitStack()
        qkvp = es_qkv.enter_context(
            tc.tile_pool(name="qkv", bufs=1, side="right"))

        bvb = consts.tile([128, c.C], BF16, name="bvb")
        nc.sync.dma_start(out=bvb, in_=broadcast_ap(bv))
        bqs = consts.tile([128, c.NPAIR], F32, name="bqs")
        nc.sync.dma_start(out=bqs, in_=bq)
        bks = consts.tile([128, c.NPAIR], F32, name="bks")
        nc.sync.dma_start(out=bks, in_=bk)
        selt = consts.tile([16, c.NPAIR, 128], BF16, name="selt")
        nc.sync.dma_start(out=selt, in_=sel)
        mb = consts.tile([128, 8], F32, name="mb")
        nc.sync.dma_start(out=mb, in_=mbias)
        ident = consts.tile([128, 128], BF16, name="ident")
        make_identity(nc, ident)
        eps_t = consts.tile([128, 1], F32, name="eps")
        nc.vector.memset(eps_t, c.eps)

        msk = mskp.tile([128, 8, c.TT], BF16, name="msk")
        nc.sync.dma_start(out=msk, in_=masks)
        x2_sb = x2p.tile([128, c.NTB, c.C], BF16, name="x2")
        attnT = attp.tile([128, c.CB, c.TOWN], BF16, name="attnT")

        KT = qkvp.tile([128, c.NPAIR, c.T2], BF16, name="KT")
        V = qkvp.tile([128, c.NSB, c.H, 66], BF16, name="V")
        QT1 = qkvp.tile([128, c.NPAIR, c.TT], BF16, name="QT1")

        es_s12 = ExitStack()
        hTp = es_s12.enter_context(tc.tile_pool(name="hTp", bufs=1))
        p1 = es_s12.enter_context(tc.tile_pool(name="p1", bufs=2))
        wstr = es_s12.enter_context(tc.tile_pool(name="wstr", bufs=1))
        QT0 = hTp.tile([128, c.NPAIR, c.TT], BF16, name="QT0")
        QTS = [QT0, QT1]
        hT_of = {}

        nc.vector.memset(V[:, :, :, 64:66], 0.0)
        nc.vector.memset(V[:, :, :, 64:65], 1.0)       # ones col (denom)
        bvv = bvb.rearrange("p (h d) -> p h d", d=64)

        def ln_tile(pool, x_t, name):
            """LayerNorm (no gamma/beta) [128, C] -> bf16 tile."""
            n_sub = c.C // 512
            stats = pool.tile([128, n_sub, 6], F32, name=f"st_{name}")
            for i in range(n_sub):
                nc.vector.bn_stats(out=stats[:, i, :],
                                   in_=x_t[:, i * 512:(i + 1) * 512])
            mv = pool.tile([128, 2], F32, name=f"mv_{name}")
            nc.vector.bn_aggr(out=mv, in_=stats)
            rstd = pool.tile([128, 1], F32, name=f"rs_{name}")
            nc.scalar.activation(out=rstd, in_=mv[:, 1:2], func=AF.Sqrt,
                                 bias=eps_t, scale=1.0)
            nc.vector.reciprocal(out=rstd, in_=rstd)
            h_t = pool.tile([128, c.C], BF16, name=f"h_{name}")
            nc.vector.tensor_scalar(out=h_t, in0=x_t, scalar1=mv[:, 0:1],
                                    scalar2=rstd, op0=ALU.subtract,
                                    op1=ALU.mult)
            return h_t

        def transpose_to(mkpst, h_t, dst, col0, name):
            """PE-transpose token-major [128, C] bf16 into feature-major
            dst[:, cb, col0:col0+128]."""
            for g0 in range(0, c.CB, 4):
                pst = mkpst(name)
                for j in range(4):
                    nc.tensor.transpose(
                        pst[:, j * 128:(j + 1) * 128],
                        h_t[:, (g0 + j) * 128:(g0 + j + 1) * 128], ident)
                nc.vector.tensor_copy(
                    out=dst[:, g0:g0 + 4, col0:col0 + 128],
                    in_=pst.rearrange("p (g t) -> p g t", g=4))

        def slab_ln(si, mkpst):
            rows0 = SLABS[si][0]
            hTs = hTp.tile([128, c.CB, c.TT], BF16, name="hTs", bufs=2)
            hT_of[si] = hTs
            for tb in range(c.TTB):
                x_t = p1.tile([128, c.C], F32, name="x_ln1")
                nc.sync.dma_start(out=x_t,
                                  in_=xp[rows0 + tb * 128:
                                         rows0 + (tb + 1) * 128, :])
                h_t = ln_tile(p1, x_t, "ln1")
                transpose_to(mkpst, h_t, hTs, tb * 128, "h1")
                yield

        def slab_kvq(si, mkps):
            rows0, sb0, qtt = SLABS[si]
            hTs = hT_of[si]
            gsl = slice(rows0, rows0 + c.TT)
            for pr in range(c.NPAIR):
                wk_t = wstr.tile([128, c.CB, 2, 64], BF16, name="wk_t",
                                 bufs=2)
                nc.sync.dma_start(out=wk_t, in_=wk[pr])
                pk = mkps("pk", [128, c.TT])
                for cb in range(c.CB):
                    nc.tensor.matmul(pk, wk_t[:, cb], hTs[:, cb, :],
                                     start=(cb == 0), stop=(cb == c.CB - 1))
                nc.vector.tensor_scalar(out=KT[:, pr, gsl], in0=pk,
                                        scalar1=bks[:, pr:pr + 1],
                                        scalar2=None, op0=ALU.add)
                yield
            for hh in range(0, c.H, c.DH):
                wv_t = wstr.tile([128, c.CB, c.DH, 64], BF16, name="wv_t",
                                 bufs=1)
                nc.sync.dma_start(out=wv_t, in_=wv[:, :, hh:hh + c.DH, :])
                for lsb in range(c.SBB):
                    pv = mkps("pv", [128, c.DH * 64])
                    for cb in range(c.CB):
                        nc.tensor.matmul(
                            pv, hTs[:, cb, lsb * 128:(lsb + 1) * 128],
                            wv_t[:, cb],
                            start=(cb == 0), stop=(cb == c.CB - 1))
                    nc.vector.tensor_tensor(
                        out=V[:, sb0 + lsb, hh:hh + c.DH, 0:64],
                        in0=pv.rearrange("p (h d) -> p h d", d=64),
                        in1=bvv[:, hh:hh + c.DH, :], op=ALU.add)
                    yield
            if qtt is not None:
                for pr in range(c.NPAIR):
                    wq_t = wstr.tile([128, c.CB, 2, 64], BF16, name="wq_t",
                                     bufs=2)
                    nc.sync.dma_start(out=wq_t, in_=wq[pr])
                    pq = mkps("pq", [128, c.TT])
                    for cb in range(c.CB):
                        nc.tensor.matmul(pq, wq_t[:, cb], hTs[:, cb, :],
                                         start=(cb == 0),
                                         stop=(cb == c.CB - 1))
                    nc.vector.tensor_scalar(out=QTS[qtt][:, pr, :], in0=pq,
                                            scalar1=bqs[:, pr:pr + 1],
                                            scalar2=None, op0=ALU.add)
                    yield

        # ================= seg1: slabs A, B + LN of C, D =================
        with ExitStack() as phA:
            psA = phA.enter_context(tc.tile_pool(name="psA", bufs=2,
                                                 space="PSUM"))
            mkpsA = lambda nm, shape: psA.tile(shape, F32, name=nm)
            mkpstA = lambda nm: psA.tile([128, 512], BF16, name="pst")
            run(slab_ln(0, mkpstA))
            run(slab_kvq(0, mkpsA))
            run(slab_ln(1, mkpstA))
            run(slab_kvq(1, mkpsA))
            run(slab_ln(2, mkpstA))
            run(slab_ln(3, mkpstA))

        # ================= attention machinery ===========================
        es_att = ExitStack()
        attw = es_att.enter_context(
            tc.tile_pool(name="attw", bufs=1, side="right"))
        ps_att = es_att.enter_context(
            tc.tile_pool(name="ps_att", bufs=1, space="PSUM"))

        def att_gen(tt):
            tsl = slice(tt * c.TT, (tt + 1) * c.TT)
            QTt = QTS[tt]
            steps = steps_tt[tt]
            ns = len(steps)
            den_all = attw.tile([16, c.TT], F32, name="den_all", bufs=2)
            for pr in range(c.NPAIR):
                pav0 = ps_att.tile([128, c.TT], F32, name="pav0")
                pav1 = ps_att.tile([128, c.TT], F32, name="pav1")
                for si, (sb, kind, idx) in enumerate(steps):
                    ssl = slice(sb * 128, (sb + 1) * 128)
                    psc = ps_att.tile([128, 2, c.TT], F32, name="psc",
                                      bufs=2)
                    nc.tensor.matmul(psc[:, 0, :], KT[0:64, pr, ssl],
                                     QTt[0:64, pr, :],
                                     start=True, stop=True)
                    nc.tensor.matmul(psc[:, 1, :], KT[64:128, pr, ssl],
                                     QTt[64:128, pr, :],
                                     start=True, stop=True)
                    pexp = attw.tile([128, 2, c.TT], BF16, name="pexp",
                                     bufs=2)
                    bias = mb[:, idx:idx + 1] if kind == 'b' else 0.0
                    nc.scalar.activation(out=pexp, in_=psc, func=AF.Exp,
                                         scale=c.scale, bias=bias)
                    if kind == 'm':
                        nc.vector.tensor_tensor(
                            out=pexp[:, 0, :], in0=pexp[:, 0, :],
                            in1=msk[:, idx, :], op=ALU.mult)
                        nc.vector.tensor_tensor(
                            out=pexp[:, 1, :], in0=pexp[:, 1, :],
                            in1=msk[:, idx, :], op=ALU.mult)
                    nc.tensor.matmul(pav0[0:65, :],
                                     V[:, sb, 2 * pr, 0:65],
                                     pexp[:, 0, :],
                                     start=(si == 0), stop=(si == ns - 1))
                    nc.tensor.matmul(pav1[0:65, :],
                                     V[:, sb, 2 * pr + 1, 0:65],
                                     pexp[:, 1, :],
                                     start=(si == 0), stop=(si == ns - 1))
                    yield
                # evacuate unnormalized values + denominators
                nc.vector.tensor_copy(out=attnT[0:64, pr, tsl],
                                      in_=pav0[0:64, :])
                stg0 = attw.tile([65, c.TT], F32, name="stg", tag="stg",
                                 bufs=2)
                nc.vector.tensor_copy(out=stg0[64:65, :],
                                      in_=pav0[64:65, :])
                nc.sync.dma_start(out=den_all[2 * pr:2 * pr + 1, :],
                                  in_=stg0[64:65, :])
                tmp1 = attw.tile([64, c.TT], BF16, name="tmp1", bufs=2)
                nc.vector.tensor_copy(out=tmp1, in_=pav1[0:64, :])
                nc.sync.dma_start(out=attnT[64:128, pr, tsl], in_=tmp1)
                stg1 = attw.tile([65, c.TT], F32, name="stg1", tag="stg",
                                 bufs=2)
                nc.vector.tensor_copy(out=stg1[64:65, :],
                                      in_=pav1[64:65, :])
                nc.sync.dma_start(out=den_all[2 * pr + 1:2 * pr + 2, :],
                                  in_=stg1[64:65, :])
                yield
            # batched normalize
            rden_f = attw.tile([16, c.TT], F32, name="rden_f", bufs=1)
            nc.vector.reciprocal(out=rden_f, in_=den_all)
            rden = attw.tile([16, c.TT], BF16, name="rden", bufs=1)
            nc.vector.tensor_copy(out=rden, in_=rden_f)
            for pr in range(c.NPAIR):
                prd = ps_att.tile([128, c.TT], F32, name="prd", tag="psc",
                                  bufs=2)
                nc.tensor.matmul(prd, selt[:, pr, :], rden,
                                 start=True, stop=True)
                nc.vector.tensor_tensor(out=attnT[:, pr, tsl],
                                        in0=attnT[:, pr, tsl],
                                        in1=prd, op=ALU.mult)
                if pr % 2:
                    yield

        # ================= seg2: att(tt0) x K/V/Q of C, D ================
        with ExitStack() as ph2:
            ps_qw = ph2.enter_context(tc.tile_pool(name="ps_qw", bufs=2,
                                                   space="PSUM"))
            mkqw = lambda nm, shape: ps_qw.tile(shape, F32, name="qw",
                                                tag="qw")

            def kvq_cd():
                yield from slab_kvq(2, mkqw)
                yield from slab_kvq(3, mkqw)

            drive((att_gen(0), 2), (kvq_cd(), 1))
        es_s12.close()       # free hT slabs, QT0, p1, wstr (left)

        # ============ seg3 pools: downstream tt0 (left side) =============
        es_s34 = ExitStack()
        rtp = es_s34.enter_context(tc.tile_pool(name="rtp", bufs=1))
        consts3 = es_s34.enter_context(tc.tile_pool(name="consts3", bufs=1))
        wpp = es_s34.enter_context(tc.tile_pool(name="wpp", bufs=1))
        p4 = es_s34.enter_context(tc.tile_pool(name="p4", bufs=2))
        w1s = es_s34.enter_context(tc.tile_pool(name="w1s", bufs=2))
        es_dw3 = ExitStack()
        dwh = {'p': es_dw3.enter_context(tc.tile_pool(name="ps_dw", bufs=2,
                                                      space="PSUM"))}
        mkdw = lambda shape, dt=F32: dwh['p'].tile(shape, dt, name="dw",
                                                   tag="dw")

        bpb = consts3.tile([128, c.C], BF16, name="bpb")
        b2b = consts3.tile([128, c.C], BF16, name="b2b")
        for t, src in [(bpb, bp), (b2b, b2)]:
            tf = consts3.tile([128, c.C], F32, name="bias_f", bufs=1)
            nc.sync.dma_start(out=tf, in_=broadcast_ap(src))
            nc.vector.tensor_copy(out=t, in_=tf)
        b1s = consts3.tile([128, c.FC], F32, name="b1s")
        nc.sync.dma_start(out=b1s, in_=b1)

        def outproj_gen(tt):
            for ch in range(c.NCH):
                csl = slice(ch * c.CHW, (ch + 1) * c.CHW)
                wp_t = wpp.tile([128, c.CB, c.CHW], BF16, name="wp_t",
                                bufs=1)
                nc.sync.dma_start(out=wp_t, in_=wp[ch])
                for ltb in range(c.TTB):
                    tb = tt * c.TTB + ltb
                    x_t = p4.tile([128, c.C], BF16, name="x_res", bufs=4)
                    nc.sync.dma_start(
                        out=x_t, in_=xpb[tb * 128:(tb + 1) * 128, :])
                    nc.vector.tensor_tensor(out=x_t, in0=x_t, in1=bpb,
                                            op=ALU.add)
                    pd = mkdw([128, c.CHW])
                    for cb in range(c.CB):
                        nc.tensor.matmul(
                            pd, attnT[:, cb, tb * 128:(tb + 1) * 128],
                            wp_t[:, cb], start=(cb == 0),
                            stop=(cb == c.CB - 1))
                    nc.vector.tensor_tensor(out=x2_sb[:, tb, csl], in0=pd,
                                            in1=x_t[:, csl], op=ALU.add)
                    yield

        def ln2_gen(tt):
            h2T = h2p.tile([128, c.CB, c.TT], BF16, name="h2T", bufs=1)
            hold['h2T'] = h2T
            mvs = p4.tile([128, c.TTB, 2], F32, name="mv2", bufs=1)
            for i in range(c.TTB):
                tb = tt * c.TTB + i
                stats = p4.tile([128, 2, 6], F32, name="st2", bufs=2)
                for j in range(2):
                    nc.vector.bn_stats(
                        out=stats[:, j, :],
                        in_=x2_sb[:, tb, j * 512:(j + 1) * 512])
                nc.vector.bn_aggr(out=mvs[:, i, :], in_=stats)
                yield
            rstds = p4.tile([128, c.TTB], F32, name="rsd2", bufs=1)
            nc.scalar.activation(out=rstds, in_=mvs[:, :, 1], func=AF.Sqrt,
                                 bias=eps_t, scale=1.0)
            nc.vector.reciprocal(out=rstds, in_=rstds)
            for i in range(c.TTB):
                tb = tt * c.TTB + i
                h2_t = p4.tile([128, c.C], BF16, name="h2t", bufs=2)
                nc.vector.tensor_scalar(out=h2_t, in0=x2_sb[:, tb, :],
                                        scalar1=mvs[:, i, 0:1],
                                        scalar2=rstds[:, i:i + 1],
                                        op0=ALU.subtract, op1=ALU.mult)
                transpose_to(lambda nm: mkdw([128, 512], BF16),
                             h2_t, hold['h2T'], i * 128, "h2")
                yield

        def ffn1_gen(tt):
            rT = rtp.tile([128, c.FC, c.TT], BF16, name="rT", bufs=1)
            hold['rT'] = rT
            h2T = hold['h2T']
            for fc in range(c.FC):
                w1_t = w1s.tile([128, c.CB, 128], BF16, name="w1_t",
                                bufs=2)
                nc.sync.dma_start(out=w1_t, in_=w1[fc])
                pf = mkdw([128, c.TT])
                for cb in range(c.CB):
                    nc.tensor.matmul(pf, w1_t[:, cb], h2T[:, cb, :],
                                     start=(cb == 0), stop=(cb == c.CB - 1))
                nc.scalar.activation(out=rT[:, fc, :], in_=pf,
                                     func=AF.Relu, bias=b1s[:, fc:fc + 1])
                yield

        hold = {}

        def down_gen(tt):
            yield from outproj_gen(tt)
            yield from ln2_gen(tt)

        # seg3: attention(tt1) x [out-proj, LN2, FFN1] of tt0
        def down0():
            yield from down_gen(0)
            yield from ffn1_gen(0)

        drive((att_gen(1), 3), (down0(), 1))
        rT0 = hold['rT']

        es_dw3.close()       # free seg3 dwork psum (above ps_att)
        es_att.close()       # free attw + ps_att
        es_qkv.close()       # free KT/V/QT1 (right)

        # ============ seg4: FFN2(tt0) x down(tt1); FFN1/FFN2(tt1) ========
        with ExitStack() as ph4:
            w2p = ph4.enter_context(
                tc.tile_pool(name="w2p", bufs=1, side="right"))
            ps4 = ph4.enter_context(tc.tile_pool(name="ps4", bufs=1,
                                                 space="PSUM"))
            dwh['p'] = ph4.enter_context(tc.tile_pool(name="ps_dw4", bufs=2,
                                                      space="PSUM"))
            w2sb = w2p.tile([128, c.FC, c.C], BF16, name="w2sb")
            for g in range(8):
                nc.sync.dma_start(out=w2sb[:, g * 4:(g + 1) * 4, :],
                                  in_=w2[:, g * 4:(g + 1) * 4, :])

            def ffn2_gen(tt, rT):
                for ch in range(c.NCH):
                    csl = slice(ch * c.CHW, (ch + 1) * c.CHW)
                    pos = [ps4.tile([128, c.CHW], F32, name=f"po{i}")
                           for i in range(c.TTB)]
                    for fb in range(c.FC):
                        for i in range(c.TTB):
                            nc.tensor.matmul(
                                pos[i], rT[:, fb, i * 128:(i + 1) * 128],
                                w2sb[:, fb, csl], start=(fb == 0),
                                stop=(fb == c.FC - 1))
                        if fb % 2:
                            yield
                    for i in range(c.TTB):
                        tb = tt * c.TTB + i
                        nc.vector.tensor_tensor(out=x2_sb[:, tb, csl],
                                                in0=pos[i],
                                                in1=x2_sb[:, tb, csl],
                                                op=ALU.add)
                        nc.vector.tensor_tensor(out=x2_sb[:, tb, csl],
                                                in0=x2_sb[:, tb, csl],
                                                in1=b2b[:, csl],
                                                op=ALU.add)
                        nc.sync.dma_start(
                            out=y[tb * 128:(tb + 1) * 128, csl],
                            in_=x2_sb[:, tb, csl])
                        yield

            drive((ffn2_gen(0, rT0), 3), (down_gen(1), 1))
            run(ffn1_gen(1))
            run(ffn2_gen(1, hold['rT']))
        es_s34.close()
    return nc


# ======================================================================
# Host side: shard full inputs across 8 cores, run the SPMD NEFF, gather.
# ======================================================================
import numpy as np
import ml_dtypes

BF = ml_dtypes.bfloat16
_STATE = {}


def core_perm(pid, T):
    """Row permutation for one core: [ownL, ownH, otherL, otherH] blocks.
    Cores 2b+0 own row-blocks (0, 3) of batch b; cores 2b+1 own (1, 2) —
    balanced causal attention load."""
    Tb = T // 4
    own, other = {0: ((0, 3), (1, 2)), 1: ((1, 2), (0, 3))}[pid]
    blocks = [own[0], own[1], other[0], other[1]]
    return np.concatenate([np.arange(b * Tb, (b + 1) * Tb) for b in blocks])


def build_masks_np(perm, T2):
    """Diagonal elementwise masks [128, 8, TT] + all-or-none biases
    [128, 8] for the far blocks."""
    TT = T2 // 4
    masks = np.zeros((8, 128, TT), np.float32)
    mbias = np.zeros((8,), np.float32)
    for sb in range(4):          # tt0 ownL diag
        tpos = np.arange(0, TT)
        spos = np.arange(sb * 128, (sb + 1) * 128)
        masks[sb] = (perm[spos][:, None] <= perm[tpos][None, :])
    for sb in range(4, 8):       # tt1 ownH diag
        tpos = np.arange(TT, 2 * TT)
        spos = np.arange(sb * 128, (sb + 1) * 128)
        masks[sb] = (perm[spos][:, None] <= perm[tpos][None, :])
    for j in range(4):           # tt0 vs othL
        tpos = np.arange(0, TT)
        spos = np.arange((8 + j) * 128, (9 + j) * 128)
        allp = (perm[spos][:, None] <= perm[tpos][None, :])
        assert allp.all() or not allp.any()
        mbias[j] = 0.0 if allp.all() else -60.0
    for j in range(4):           # tt1 vs othH
        tpos = np.arange(TT, 2 * TT)
        spos = np.arange((12 + j) * 128, (13 + j) * 128)
        allp = (perm[spos][:, None] <= perm[tpos][None, :])
        assert allp.all() or not allp.any()
        mbias[4 + j] = 0.0 if allp.all() else -60.0
    m = np.ascontiguousarray(masks.transpose(1, 0, 2)).astype(BF)
    mbt = np.ascontiguousarray(
        np.broadcast_to(mbias[None, :], (128, 8))).astype(np.float32)
    return m, mbt


def prep_weights(inputs, cfg):
    """Fold LN gammas/betas into following projections, pre-tile, bf16."""
    c = cfg
    f32 = lambda n: np.asarray(inputs[n], dtype=np.float32)
    g1, be1 = f32('g1'), f32('be1')
    g2, be2 = f32('g2'), f32('be2')
    wq, wk, wv = f32('wq'), f32('wk'), f32('wv')
    w = {}
    for nm, wt, bt in [('q', wq, f32('bq')), ('k', wk, f32('bk'))]:
        we = wt * g1[None, :, None]
        be = np.einsum('c,hcd->hd', be1, wt) + bt
        w['w' + nm] = np.ascontiguousarray(
            we.reshape(c.NPAIR, 2, c.CB, 128, 64)
              .transpose(0, 3, 2, 1, 4)).astype(BF)
        w['b' + nm] = np.ascontiguousarray(
            be.reshape(c.NPAIR, 2, 64).transpose(1, 2, 0)
              .reshape(128, c.NPAIR)).astype(np.float32)
    wve = wv * g1[None, :, None]
    bve = np.einsum('c,hcd->hd', be1, wv) + f32('bv')
    w['wv'] = np.ascontiguousarray(
        wve.reshape(c.H, c.CB, 128, 64).transpose(2, 1, 0, 3)).astype(BF)
    w['bv'] = np.ascontiguousarray(bve.reshape(-1)).astype(BF)
    wp = f32('wp')
    w['wp'] = np.ascontiguousarray(
        wp.reshape(c.CB, 128, c.NCH, c.CHW).transpose(2, 1, 0, 3)
    ).astype(BF)
    w['bp'] = f32('bp')
    w1 = f32('w1')
    w1e = w1 * g2[:, None]
    b1e = be2 @ w1 + f32('b1')
    w['w1'] = np.ascontiguousarray(
        w1e.reshape(c.CB, 128, c.FC, 128).transpose(2, 1, 0, 3)).astype(BF)
    w['b1'] = np.ascontiguousarray(
        b1e.reshape(c.FC, 128).T).astype(np.float32)
    w['w2'] = np.ascontiguousarray(
        f32('w2').reshape(c.FC, 128, c.C).transpose(1, 0, 2)).astype(BF)
    w['b2'] = f32('b2')
    selm = np.zeros((16, c.NPAIR, 128), np.float32)
    for pr in range(c.NPAIR):
        selm[2 * pr, pr, 0:64] = 1.0
        selm[2 * pr + 1, pr, 64:128] = 1.0
    w['sel'] = selm.astype(BF)
    return w


def get_compiled():
    if 'nc' in _STATE:
        return _STATE['nc'], _STATE['cfg']
    import concourse.bacc as bacc
    cfg = Cfg()
    nc = bacc.Bacc("TRN2", target_bir_lowering=False, debug=False,
                   num_devices=8)
    build(nc, cfg)
    nc.compile()
    _STATE['nc'], _STATE['cfg'] = nc, cfg
    return nc, cfg


def kernel(**inputs):
    from concourse import bass_utils
    x = np.ascontiguousarray(np.asarray(inputs['x'], dtype=np.float32))
    B, T, C = x.shape
    nc, cfg = get_compiled()
    w = prep_weights(inputs, cfg)
    in_maps = []
    perms = []
    for core in range(8):
        b, pid = core // 2, core % 2
        perm = core_perm(pid, T)
        perms.append((b, perm))
        m, mbt = build_masks_np(perm, T)
        im = dict(w)
        xc = np.ascontiguousarray(x[b][perm])
        im['xp'] = xc
        im['xpb'] = xc.astype(BF)
        im['masks'] = m
        im['mbias'] = mbt
        in_maps.append(im)
    res = bass_utils.run_bass_kernel_spmd(nc, in_maps,
                                          core_ids=list(range(8)),
                                          **_STATE.get('run_kwargs', {}))
    y = np.zeros((B, T, C), np.float32)
    for core in range(8):
        b, perm = perms[core]
        y[b][perm[:T // 2]] = res.results[core]['y'].astype(np.float32)
    _STATE['last_result'] = res
    return y


# revision 10
# speedup vs baseline: 1.0419x; 1.0419x over previous
import sys
if '/opt/trn_rl_repo' not in sys.path:
    sys.path.insert(0, '/opt/trn_rl_repo')
"""Bass/Tile kernel for one transformer block, uniform SPMD program,
software-pipelined across the two query t-tiles.

Each core receives row-permuted inputs (own TOWN rows first) plus
data-driven causal masks/biases, so all 8 cores run the identical NEFF.

Macro-pipeline (emission order == approximate execution order):
  seg1: slabs A,B (LN1+h^T, K, V, Q-tt0)            [PE dense]
  LN1 for slabs C,D (keeps all ACT Sqrts adjacent)
  seg2: attention(tt0)  x  K/V/Q projections of C,D  [exp hides under PE]
  seg3: attention(tt1)  x  out-proj/LN2/FFN1 of tt0
  seg4: FFN2(tt0) x out-proj/LN2(tt1); FFN1(tt1); FFN2(tt1)

LayerNorm gammas/betas are folded into the following projection weights
on the host; weights ship pre-tiled bf16.  Softmax normalization is
deferred: outputs are written unnormalized, per-head denominators are
gathered into one [16, TT] tile, reciprocal'd once, and broadcast back
over each head's 64 feature rows with one small matmul per pair.
Fully-masked / fully-passing far blocks use a per-core additive bias
inside the exp instead of elementwise masks.
"""
from contextlib import ExitStack

import concourse.bass as bass
import concourse.mybir as mybir
import concourse.tile as tile
from concourse.masks import make_identity

F32 = mybir.dt.float32
BF16 = mybir.dt.bfloat16
AF = mybir.ActivationFunctionType
ALU = mybir.AluOpType


class Cfg:
    def __init__(self, T2=2048, C=1024, H=16, F=None, eps=1e-5):
        self.T2, self.C, self.H = T2, C, H
        self.F = 4 * C if F is None else F
        self.HS = 64
        self.eps = eps
        self.TBLK = T2 // 4          # row block (ownL/ownH/otherL/otherH)
        self.TOWN = 2 * self.TBLK    # rows this core owns
        self.TT = self.TBLK          # t-tile width == block
        self.NTT = 2
        self.CB = C // 128
        self.NPAIR = H // 2
        self.NSB = T2 // 128         # s-blocks
        self.SBB = self.TBLK // 128  # s-blocks per row-block
        self.NTB = self.TOWN // 128  # own token-blocks
        self.TTB = self.TT // 128    # token-blocks per t-tile
        self.FC = self.F // 128
        self.DH = min(H, 8)          # heads per V-proj chunk
        self.NCH = max(C // 512, 1)  # c_out chunks (FFN2 + out-proj)
        self.CHW = min(C, 512)
        self.scale = C ** -0.5

    def att_steps(self):
        """Per t-tile: list of (sb, kind, idx).  kind: 'm' = elementwise
        mask msk[idx], 'b' = additive exp-bias mb[:, idx], None = pass."""
        s0 = ([(sb, 'm', sb) for sb in range(4)]            # ownL diag
              + [(8 + j, 'b', j) for j in range(4)])        # othL all/none
        s1 = ([(sb, None, 0) for sb in range(4)]            # ownL pass
              + [(sb, 'm', sb) for sb in range(4, 8)]       # ownH diag
              + [(8 + j, None, 0) for j in range(4)]        # othL pass
              + [(12 + j, 'b', 4 + j) for j in range(4)])   # othH all/none
        return [s0, s1]


def broadcast_ap(ap, parts=128):
    """[N] vector AP -> [parts, N] partition-broadcast AP (DMA source)."""
    return bass.AP(tensor=ap.tensor, offset=ap.offset,
                   ap=[[0, parts]] + list(ap.ap))


def drive(*streams):
    """Interleave generators: streams = (gen, weight) pairs; each round
    advances gen by `weight` yields until all are exhausted."""
    live = [[g, w] for g, w in streams]
    while live:
        for gw in list(live):
            g, w = gw
            try:
                for _ in range(w):
                    next(g)
            except StopIteration:
                live.remove(gw)


def run(g):
    for _ in g:
        pass


def build(nc, cfg: Cfg):
    c = cfg
    steps_tt = c.att_steps()

    def din(name, shape, dt=F32):
        return nc.dram_tensor(name, shape, dt, kind="ExternalInput").ap()

    xp = din("xp", [c.T2, c.C])
    xpb = din("xpb", [c.T2, c.C], BF16)
    masks = din("masks", [128, 8, c.TT], BF16)
    mbias = din("mbias", [128, 8])
    sel = din("sel", [16, c.NPAIR, 128], BF16)
    wq = din("wq", [c.NPAIR, 128, c.CB, 2, 64], BF16)
    wk = din("wk", [c.NPAIR, 128, c.CB, 2, 64], BF16)
    wv = din("wv", [128, c.CB, c.H, 64], BF16)
    bq = din("bq", [128, c.NPAIR])
    bk = din("bk", [128, c.NPAIR])
    bv = din("bv", [c.C], BF16)
    wp = din("wp", [c.NCH, 128, c.CB, c.CHW], BF16)
    bp = din("bp", [c.C], BF16)
    w1 = din("w1", [c.FC, 128, c.CB, 128], BF16)
    b1 = din("b1", [128, c.FC])
    w2 = din("w2", [128, c.FC, c.C], BF16)
    b2 = din("b2", [c.C], BF16)
    y = nc.dram_tensor("y", [c.TOWN, c.C], BF16, kind="ExternalOutput").ap()

    # slabs: (perm-row base, first s-block, q t-tile | None)
    SLABS = [(0, 0, 0), (1024, 8, None), (512, 4, 1), (1536, 12, None)]

    with tile.TileContext(nc) as tc:
      with ExitStack() as top:
        # LEFT:  consts | msk | x2 | attnT | h2T | [hT+QT0, p1, wstr
        #        (seg1-2)] -> [rT, consts3, wpp, p4, w1s (seg3-4)]
        # RIGHT: qkv(KT/V/QT1) | attw (seg2-3) -> w2p (seg4)
        consts = top.enter_context(tc.tile_pool(name="consts", bufs=1))
        mskp = top.enter_context(tc.tile_pool(name="mskp", bufs=1))
        x2p = top.enter_context(tc.tile_pool(name="x2p", bufs=1))
        attp = top.enter_context(tc.tile_pool(name="attp", bufs=1))
        h2p = top.enter_context(tc.tile_pool(name="h2p", bufs=1))
        es_qkv = ExitStack()
        qkvp = es_qkv.enter_context(
            tc.tile_pool(name="qkv", bufs=1, side="right"))

        bvb = consts.tile([128, c.C], BF16, name="bvb")
        nc.sync.dma_start(out=bvb, in_=broadcast_ap(bv))
        bqs = consts.tile([128, c.NPAIR], F32, name="bqs")
        nc.sync.dma_start(out=bqs, in_=bq)
        bks = consts.tile([128, c.NPAIR], F32, name="bks")
        nc.sync.dma_start(out=bks, in_=bk)
        selt = consts.tile([16, c.NPAIR, 128], BF16, name="selt")
        mb = consts.tile([128, 8], F32, name="mb")
        ident = consts.tile([128, 128], BF16, name="ident")
        make_identity(nc, ident)
        eps_t = consts.tile([128, 1], F32, name="eps")
        nc.vector.memset(eps_t, c.eps)

        bpb = consts.tile([128, c.C], BF16, name="bpb")
        nc.sync.dma_start(out=bpb, in_=broadcast_ap(bp))
        b2b = consts.tile([128, c.C], BF16, name="b2b")
        nc.sync.dma_start(out=b2b, in_=broadcast_ap(b2))
        b1s = consts.tile([128, c.FC], F32, name="b1s")
        nc.sync.dma_start(out=b1s, in_=b1)

        msk = mskp.tile([128, 8, c.TT], BF16, name="msk")
        x2_sb = x2p.tile([128, c.NTB, c.C], BF16, name="x2")
        attnT = attp.tile([128, c.CB, c.TOWN], BF16, name="attnT")

        KT = qkvp.tile([128, c.NPAIR, c.T2], BF16, name="KT")
        V = qkvp.tile([128, c.NSB, c.H, 66], BF16, name="V")
        QT1 = qkvp.tile([128, c.NPAIR, c.TT], BF16, name="QT1")

        es_s12 = ExitStack()
        hTp = es_s12.enter_context(tc.tile_pool(name="hTp", bufs=1))
        p1 = es_s12.enter_context(tc.tile_pool(name="p1", bufs=2))
        wstr = es_s12.enter_context(tc.tile_pool(name="wstr", bufs=1))
        QT0 = hTp.tile([128, c.NPAIR, c.TT], BF16, name="QT0")
        QTS = [QT0, QT1]
        hT_of = {}

        nc.vector.memset(V[:, :, :, 64:66], 0.0)
        nc.vector.memset(V[:, :, :, 64:65], 1.0)       # ones col (denom)
        bvv = bvb.rearrange("p (h d) -> p h d", d=64)

        def ln_tile(pool, x_t, name):
            """LayerNorm (no gamma/beta) [128, C] -> bf16 tile."""
            n_sub = c.C // 512
            stats = pool.tile([128, n_sub, 6], F32, name=f"st_{name}")
            for i in range(n_sub):
                nc.vector.bn_stats(out=stats[:, i, :],
                                   in_=x_t[:, i * 512:(i + 1) * 512])
            mv = pool.tile([128, 2], F32, name=f"mv_{name}")
            nc.vector.bn_aggr(out=mv, in_=stats)
            rstd = pool.tile([128, 1], F32, name=f"rs_{name}")
            nc.scalar.activation(out=rstd, in_=mv[:, 1:2], func=AF.Sqrt,
                                 bias=eps_t, scale=1.0)
            nc.vector.reciprocal(out=rstd, in_=rstd)
            h_t = pool.tile([128, c.C], BF16, name=f"h_{name}")
            nc.vector.tensor_scalar(out=h_t, in0=x_t, scalar1=mv[:, 0:1],
                                    scalar2=rstd, op0=ALU.subtract,
                                    op1=ALU.mult)
            return h_t

        def transpose_to(mkpst, h_t, dst, col0, name):
            """PE-transpose token-major [128, C] bf16 into feature-major
            dst[:, cb, col0:col0+128]."""
            for g0 in range(0, c.CB, 4):
                pst = mkpst(name)
                for j in range(4):
                    nc.tensor.transpose(
                        pst[:, j * 128:(j + 1) * 128],
                        h_t[:, (g0 + j) * 128:(g0 + j + 1) * 128], ident)
                nc.vector.tensor_copy(
                    out=dst[:, g0:g0 + 4, col0:col0 + 128],
                    in_=pst.rearrange("p (g t) -> p g t", g=4))

        def slab_ln(si, mkpst, mkps):
            rows0, sb0, _ = SLABS[si]
            hTs = hTp.tile([128, c.CB, c.TT], BF16, name="hTs", bufs=2)
            hT_of[si] = hTs
            wv_t = wstr.tile([128, c.CB, c.DH, 64], BF16, name="wv_t",
                             bufs=1)
            nc.sync.dma_start(out=wv_t, in_=wv[:, :, 0:c.DH, :])
            for tb in range(c.TTB):
                x_t = p1.tile([128, c.C], F32, name="x_ln1")
                nc.sync.dma_start(out=x_t,
                                  in_=xp[rows0 + tb * 128:
                                         rows0 + (tb + 1) * 128, :])
                h_t = ln_tile(p1, x_t, "ln1")
                transpose_to(mkpst, h_t, hTs, tb * 128, "h1")
                pv = mkps("pv", [128, c.DH * 64])
                for cb in range(c.CB):
                    nc.tensor.matmul(
                        pv, hTs[:, cb, tb * 128:(tb + 1) * 128],
                        wv_t[:, cb],
                        start=(cb == 0), stop=(cb == c.CB - 1))
                nc.vector.tensor_tensor(
                    out=V[:, sb0 + tb, 0:c.DH, 0:64],
                    in0=pv.rearrange("p (h d) -> p h d", d=64),
                    in1=bvv[:, 0:c.DH, :], op=ALU.add)
                yield

        def slab_kvq(si, mkps):
            rows0, sb0, qtt = SLABS[si]
            hTs = hT_of[si]
            gsl = slice(rows0, rows0 + c.TT)
            for pr in range(c.NPAIR):
                wk_t = wstr.tile([128, c.CB, 2, 64], BF16, name="wk_t",
                                 bufs=2)
                nc.sync.dma_start(out=wk_t, in_=wk[pr])
                pk = mkps("pk", [128, c.TT])
                for cb in range(c.CB):
                    nc.tensor.matmul(pk, wk_t[:, cb], hTs[:, cb, :],
                                     start=(cb == 0), stop=(cb == c.CB - 1))
                nc.vector.tensor_scalar(out=KT[:, pr, gsl], in0=pk,
                                        scalar1=bks[:, pr:pr + 1],
                                        scalar2=None, op0=ALU.add)
                yield
            for hh in range(c.DH, c.H, c.DH):
                wv_t = wstr.tile([128, c.CB, c.DH, 64], BF16, name="wv_t",
                                 bufs=1)
                nc.sync.dma_start(out=wv_t, in_=wv[:, :, hh:hh + c.DH, :])
                for lsb in range(c.SBB):
                    pv = mkps("pv", [128, c.DH * 64])
                    for cb in range(c.CB):
                        nc.tensor.matmul(
                            pv, hTs[:, cb, lsb * 128:(lsb + 1) * 128],
                            wv_t[:, cb],
                            start=(cb == 0), stop=(cb == c.CB - 1))
                    nc.vector.tensor_tensor(
                        out=V[:, sb0 + lsb, hh:hh + c.DH, 0:64],
                        in0=pv.rearrange("p (h d) -> p h d", d=64),
                        in1=bvv[:, hh:hh + c.DH, :], op=ALU.add)
                    yield
            if qtt is not None:
                for pr in range(c.NPAIR):
                    wq_t = wstr.tile([128, c.CB, 2, 64], BF16, name="wq_t",
                                     bufs=2)
                    nc.sync.dma_start(out=wq_t, in_=wq[pr])
                    pq = mkps("pq", [128, c.TT])
                    for cb in range(c.CB):
                        nc.tensor.matmul(pq, wq_t[:, cb], hTs[:, cb, :],
                                         start=(cb == 0),
                                         stop=(cb == c.CB - 1))
                    nc.vector.tensor_scalar(out=QTS[qtt][:, pr, :], in0=pq,
                                            scalar1=bqs[:, pr:pr + 1],
                                            scalar2=None, op0=ALU.add)
                    yield

        # ================= seg1: slabs A, B + LN of C, D =================
        with ExitStack() as phA:
            psA = phA.enter_context(tc.tile_pool(name="psA", bufs=2,
                                                 space="PSUM"))
            mkpsA = lambda nm, shape: psA.tile(shape, F32, name=nm)
            mkpstA = lambda nm: psA.tile([128, 512], BF16, name="pst")
            run(slab_ln(0, mkpstA, mkpsA))
            run(slab_kvq(0, mkpsA))
            run(slab_ln(1, mkpstA, mkpsA))
            run(slab_kvq(1, mkpsA))
            run(slab_ln(2, mkpstA, mkpsA))
            run(slab_ln(3, mkpstA, mkpsA))

        # late-loaded attention constants (kept off the startup DMA path)
        nc.sync.dma_start(out=msk, in_=masks)
        nc.sync.dma_start(out=selt, in_=sel)
        nc.sync.dma_start(out=mb, in_=mbias)

        # ================= attention machinery ===========================
        es_att = ExitStack()
        attw = es_att.enter_context(
            tc.tile_pool(name="attw", bufs=1, side="right"))
        ps_att = es_att.enter_context(
            tc.tile_pool(name="ps_att", bufs=1, space="PSUM"))

        hold = {}

        def att_gen(tt):
            tsl = slice(tt * c.TT, (tt + 1) * c.TT)
            QTt = QTS[tt]
            steps = steps_tt[tt]
            ns = len(steps)
            den_all = attw.tile([16, c.TT], F32, name="den_all", bufs=2)
            hold[f'den{tt}'] = den_all
            for pr in range(c.NPAIR):
                pav0 = ps_att.tile([128, c.TT], F32, name="pav0")
                pav1 = ps_att.tile([128, c.TT], F32, name="pav1")
                for si, (sb, kind, idx) in enumerate(steps):
                    ssl = slice(sb * 128, (sb + 1) * 128)
                    psc = ps_att.tile([128, 2, c.TT], F32, name="psc",
                                      bufs=2)
                    nc.tensor.matmul(psc[:, 0, :], KT[0:64, pr, ssl],
                                     QTt[0:64, pr, :],
                                     start=True, stop=True)
                    nc.tensor.matmul(psc[:, 1, :], KT[64:128, pr, ssl],
                                     QTt[64:128, pr, :],
                                     start=True, stop=True)
                    pexp = attw.tile([128, 2, c.TT], BF16, name="pexp",
                                     bufs=2)
                    bias = mb[:, idx:idx + 1] if kind == 'b' else 0.0
                    nc.scalar.activation(out=pexp, in_=psc, func=AF.Exp,
                                         scale=c.scale, bias=bias)
                    if kind == 'm':
                        nc.vector.tensor_tensor(
                            out=pexp[:, 0, :], in0=pexp[:, 0, :],
                            in1=msk[:, idx, :], op=ALU.mult)
                        nc.vector.tensor_tensor(
                            out=pexp[:, 1, :], in0=pexp[:, 1, :],
                            in1=msk[:, idx, :], op=ALU.mult)
                    nc.tensor.matmul(pav0[0:65, :],
                                     V[:, sb, 2 * pr, 0:65],
                                     pexp[:, 0, :],
                                     start=(si == 0), stop=(si == ns - 1))
                    nc.tensor.matmul(pav1[0:65, :],
                                     V[:, sb, 2 * pr + 1, 0:65],
                                     pexp[:, 1, :],
                                     start=(si == 0), stop=(si == ns - 1))
                    yield
                # evacuate unnormalized values + denominators
                nc.vector.tensor_copy(out=attnT[0:64, pr, tsl],
                                      in_=pav0[0:64, :])
                stg0 = attw.tile([65, c.TT], F32, name="stg", tag="stg",
                                 bufs=2)
                nc.vector.tensor_copy(out=stg0[64:65, :],
                                      in_=pav0[64:65, :])
                nc.sync.dma_start(out=den_all[2 * pr:2 * pr + 1, :],
                                  in_=stg0[64:65, :])
                tmp1 = attw.tile([64, c.TT], BF16, name="tmp1", bufs=2)
                nc.vector.tensor_copy(out=tmp1, in_=pav1[0:64, :])
                nc.sync.dma_start(out=attnT[64:128, pr, tsl], in_=tmp1)
                stg1 = attw.tile([65, c.TT], F32, name="stg1", tag="stg",
                                 bufs=2)
                nc.vector.tensor_copy(out=stg1[64:65, :],
                                      in_=pav1[64:65, :])
                nc.sync.dma_start(out=den_all[2 * pr + 1:2 * pr + 2, :],
                                  in_=stg1[64:65, :])
                yield

        def norm_gen(tt):
            tsl = slice(tt * c.TT, (tt + 1) * c.TT)
            den_all = hold[f'den{tt}']
            rden_f = attw.tile([16, c.TT], F32, name="rden_f", bufs=1)
            nc.vector.reciprocal(out=rden_f, in_=den_all)
            rden = attw.tile([16, c.TT], BF16, name="rden", bufs=1)
            nc.vector.tensor_copy(out=rden, in_=rden_f)
            for pr in range(c.NPAIR):
                prd = ps_att.tile([128, c.TT], F32, name="prd", tag="psc",
                                  bufs=2)
                nc.tensor.matmul(prd, selt[:, pr, :], rden,
                                 start=True, stop=True)
                nc.vector.tensor_tensor(out=attnT[:, pr, tsl],
                                        in0=attnT[:, pr, tsl],
                                        in1=prd, op=ALU.mult)
                if pr % 2:
                    yield

        # ================= seg2: att(tt0) x K/V/Q of C, D ================
        with ExitStack() as ph2:
            ps_qw = ph2.enter_context(tc.tile_pool(name="ps_qw", bufs=2,
                                                   space="PSUM"))
            mkqw = lambda nm, shape: ps_qw.tile(shape, F32, name="qw",
                                                tag="qw")

            def kvq_cd():
                yield from slab_kvq(2, mkqw)
                yield from slab_kvq(3, mkqw)

            drive((att_gen(0), 2), (kvq_cd(), 1))
        es_s12.close()       # free hT slabs, QT0, p1, wstr (left)

        # ============ seg3 pools: downstream tt0 (left side) =============
        es_s34 = ExitStack()
        rtp = es_s34.enter_context(tc.tile_pool(name="rtp", bufs=1))
        wpp = es_s34.enter_context(tc.tile_pool(name="wpp", bufs=1))
        p4 = es_s34.enter_context(tc.tile_pool(name="p4", bufs=2))
        w1s = es_s34.enter_context(tc.tile_pool(name="w1s", bufs=2))
        es_dw3 = ExitStack()
        dwh = {'p': es_dw3.enter_context(tc.tile_pool(name="ps_dw", bufs=2,
                                                      space="PSUM"))}
        mkdw = lambda shape, dt=F32: dwh['p'].tile(shape, dt, name="dw",
                                                   tag="dw")

        def outproj_gen(tt):
            for ch in range(c.NCH):
                csl = slice(ch * c.CHW, (ch + 1) * c.CHW)
                wp_t = wpp.tile([128, c.CB, c.CHW], BF16, name="wp_t",
                                bufs=1)
                nc.sync.dma_start(out=wp_t, in_=wp[ch])
                for ltb in range(c.TTB):
                    tb = tt * c.TTB + ltb
                    x_t = p4.tile([128, c.C], BF16, name="x_res", bufs=4)
                    nc.sync.dma_start(
                        out=x_t, in_=xpb[tb * 128:(tb + 1) * 128, :])
                    nc.vector.tensor_tensor(out=x_t, in0=x_t, in1=bpb,
                                            op=ALU.add)
                    pd = mkdw([128, c.CHW])
                    for cb in range(c.CB):
                        nc.tensor.matmul(
                            pd, attnT[:, cb, tb * 128:(tb + 1) * 128],
                            wp_t[:, cb], start=(cb == 0),
                            stop=(cb == c.CB - 1))
                    nc.vector.tensor_tensor(out=x2_sb[:, tb, csl], in0=pd,
                                            in1=x_t[:, csl], op=ALU.add)
                    yield

        def ln2_gen(tt):
            h2T = h2p.tile([128, c.CB, c.TT], BF16, name="h2T", bufs=1)
            hold['h2T'] = h2T
            mvs = p4.tile([128, c.TTB, 2], F32, name="mv2", bufs=1)
            for i in range(c.TTB):
                tb = tt * c.TTB + i
                stats = p4.tile([128, 2, 6], F32, name="st2", bufs=2)
                for j in range(2):
                    nc.vector.bn_stats(
                        out=stats[:, j, :],
                        in_=x2_sb[:, tb, j * 512:(j + 1) * 512])
                nc.vector.bn_aggr(out=mvs[:, i, :], in_=stats)
                yield
            rstds = p4.tile([128, c.TTB], F32, name="rsd2", bufs=1)
            nc.scalar.activation(out=rstds, in_=mvs[:, :, 1], func=AF.Sqrt,
                                 bias=eps_t, scale=1.0)
            nc.vector.reciprocal(out=rstds, in_=rstds)
            for i in range(c.TTB):
                tb = tt * c.TTB + i
                h2_t = p4.tile([128, c.C], BF16, name="h2t", bufs=2)
                nc.vector.tensor_scalar(out=h2_t, in0=x2_sb[:, tb, :],
                                        scalar1=mvs[:, i, 0:1],
                                        scalar2=rstds[:, i:i + 1],
                                        op0=ALU.subtract, op1=ALU.mult)
                transpose_to(lambda nm: mkdw([128, 512], BF16),
                             h2_t, hold['h2T'], i * 128, "h2")
                yield

        def ffn1_gen(tt):
            rT = rtp.tile([128, c.FC, c.TT], BF16, name="rT", bufs=1)
            hold['rT'] = rT
            h2T = hold['h2T']
            for fc in range(c.FC):
                w1_t = w1s.tile([128, c.CB, 128], BF16, name="w1_t",
                                bufs=2)
                nc.sync.dma_start(out=w1_t, in_=w1[fc])
                pf = mkdw([128, c.TT])
                for cb in range(c.CB):
                    nc.tensor.matmul(pf, w1_t[:, cb], h2T[:, cb, :],
                                     start=(cb == 0), stop=(cb == c.CB - 1))
                nc.scalar.activation(out=rT[:, fc, :], in_=pf,
                                     func=AF.Relu, bias=b1s[:, fc:fc + 1])
                yield

        def down_gen(tt):
            yield from outproj_gen(tt)
            yield from ln2_gen(tt)

        # seg3: attention(tt1) x [normalize(tt0), out-proj, LN2, FFN1] tt0
        def down0():
            yield from norm_gen(0)
            yield from down_gen(0)
            yield from ffn1_gen(0)

        drive((att_gen(1), 3), (down0(), 1))
        run(norm_gen(1))
        rT0 = hold['rT']

        es_dw3.close()       # free seg3 dwork psum (above ps_att)
        es_att.close()       # free attw + ps_att
        es_qkv.close()       # free KT/V/QT1 (right)

        # ============ seg4: FFN2(tt0) x down(tt1); FFN1/FFN2(tt1) ========
        with ExitStack() as ph4:
            w2p = ph4.enter_context(
                tc.tile_pool(name="w2p", bufs=1, side="right"))
            ps4 = ph4.enter_context(tc.tile_pool(name="ps4", bufs=1,
                                                 space="PSUM"))
            dwh['p'] = ph4.enter_context(tc.tile_pool(name="ps_dw4", bufs=2,
                                                      space="PSUM"))
            w2sb = w2p.tile([128, c.FC, c.C], BF16, name="w2sb")
            for g in range(8):
                nc.sync.dma_start(out=w2sb[:, g * 4:(g + 1) * 4, :],
                                  in_=w2[:, g * 4:(g + 1) * 4, :])

            def ffn2_gen(tt, rT):
                for ch in range(c.NCH):
                    csl = slice(ch * c.CHW, (ch + 1) * c.CHW)
                    pos = [ps4.tile([128, c.CHW], F32, name=f"po{i}")
                           for i in range(c.TTB)]
                    for fb in range(c.FC):
                        for i in range(c.TTB):
                            nc.tensor.matmul(
                                pos[i], rT[:, fb, i * 128:(i + 1) * 128],
                                w2sb[:, fb, csl], start=(fb == 0),
                                stop=(fb == c.FC - 1))
                        if fb % 2:
                            yield
                    for i in range(c.TTB):
                        tb = tt * c.TTB + i
                        nc.vector.tensor_tensor(out=x2_sb[:, tb, csl],
                                                in0=pos[i],
                                                in1=x2_sb[:, tb, csl],
                                                op=ALU.add)
                        nc.vector.tensor_tensor(out=x2_sb[:, tb, csl],
                                                in0=x2_sb[:, tb, csl],
                                                in1=b2b[:, csl],
                                                op=ALU.add)
                        nc.sync.dma_start(
                            out=y[tb * 128:(tb + 1) * 128, csl],
                            in_=x2_sb[:, tb, csl])
                        yield

            drive((ffn2_gen(0, rT0), 3), (down_gen(1), 1))
            g1 = ffn1_gen(1)
            next(g1)
            next(g1)
            drive((g1, 2), (ffn2_gen(1, hold['rT']), 1))
        es_s34.close()
    return nc


# ======================================================================
# Host side: shard full inputs across 8 cores, run the SPMD NEFF, gather.
# ======================================================================
import numpy as np
import ml_dtypes

BF = ml_dtypes.bfloat16
_STATE = {}


def core_perm(pid, T):
    """Row permutation for one core: [ownL, ownH, otherL, otherH] blocks.
    Cores 2b+0 own row-blocks (0, 3) of batch b; cores 2b+1 own (1, 2) —
    balanced causal attention load."""
    Tb = T // 4
    own, other = {0: ((0, 3), (1, 2)), 1: ((1, 2), (0, 3))}[pid]
    blocks = [own[0], own[1], other[0], other[1]]
    return np.concatenate([np.arange(b * Tb, (b + 1) * Tb) for b in blocks])


def build_masks_np(perm, T2):
    """Diagonal elementwise masks [128, 8, TT] + all-or-none biases
    [128, 8] for the far blocks."""
    TT = T2 // 4
    masks = np.zeros((8, 128, TT), np.float32)
    mbias = np.zeros((8,), np.float32)
    for sb in range(4):          # tt0 ownL diag
        tpos = np.arange(0, TT)
        spos = np.arange(sb * 128, (sb + 1) * 128)
        masks[sb] = (perm[spos][:, None] <= perm[tpos][None, :])
    for sb in range(4, 8):       # tt1 ownH diag
        tpos = np.arange(TT, 2 * TT)
        spos = np.arange(sb * 128, (sb + 1) * 128)
        masks[sb] = (perm[spos][:, None] <= perm[tpos][None, :])
    for j in range(4):           # tt0 vs othL
        tpos = np.arange(0, TT)
        spos = np.arange((8 + j) * 128, (9 + j) * 128)
        allp = (perm[spos][:, None] <= perm[tpos][None, :])
        assert allp.all() or not allp.any()
        mbias[j] = 0.0 if allp.all() else -60.0
    for j in range(4):           # tt1 vs othH
        tpos = np.arange(TT, 2 * TT)
        spos = np.arange((12 + j) * 128, (13 + j) * 128)
        allp = (perm[spos][:, None] <= perm[tpos][None, :])
        assert allp.all() or not allp.any()
        mbias[4 + j] = 0.0 if allp.all() else -60.0
    m = np.ascontiguousarray(masks.transpose(1, 0, 2)).astype(BF)
    mbt = np.ascontiguousarray(
        np.broadcast_to(mbias[None, :], (128, 8))).astype(np.float32)
    return m, mbt


def prep_weights(inputs, cfg):
    """Fold LN gammas/betas into following projections, pre-tile, bf16."""
    c = cfg
    f32 = lambda n: np.asarray(inputs[n], dtype=np.float32)
    g1, be1 = f32('g1'), f32('be1')
    g2, be2 = f32('g2'), f32('be2')
    wq, wk, wv = f32('wq'), f32('wk'), f32('wv')
    w = {}
    for nm, wt, bt in [('q', wq, f32('bq')), ('k', wk, f32('bk'))]:
        we = wt * g1[None, :, None]
        be = np.einsum('c,hcd->hd', be1, wt) + bt
        w['w' + nm] = np.ascontiguousarray(
            we.reshape(c.NPAIR, 2, c.CB, 128, 64)
              .transpose(0, 3, 2, 1, 4)).astype(BF)
        w['b' + nm] = np.ascontiguousarray(
            be.reshape(c.NPAIR, 2, 64).transpose(1, 2, 0)
              .reshape(128, c.NPAIR)).astype(np.float32)
    wve = wv * g1[None, :, None]
    bve = np.einsum('c,hcd->hd', be1, wv) + f32('bv')
    w['wv'] = np.ascontiguousarray(
        wve.reshape(c.H, c.CB, 128, 64).transpose(2, 1, 0, 3)).astype(BF)
    w['bv'] = np.ascontiguousarray(bve.reshape(-1)).astype(BF)
    wp = f32('wp')
    w['wp'] = np.ascontiguousarray(
        wp.reshape(c.CB, 128, c.NCH, c.CHW).transpose(2, 1, 0, 3)
    ).astype(BF)
    w['bp'] = f32('bp')
    w1 = f32('w1')
    w1e = w1 * g2[:, None]
    b1e = be2 @ w1 + f32('b1')
    w['w1'] = np.ascontiguousarray(
        w1e.reshape(c.CB, 128, c.FC, 128).transpose(2, 1, 0, 3)).astype(BF)
    w['b1'] = np.ascontiguousarray(
        b1e.reshape(c.FC, 128).T).astype(np.float32)
    w['w2'] = np.ascontiguousarray(
        f32('w2').reshape(c.FC, 128, c.C).transpose(1, 0, 2)).astype(BF)
    w['b2'] = f32('b2')
    selm = np.zeros((16, c.NPAIR, 128), np.float32)
    for pr in range(c.NPAIR):
        selm[2 * pr, pr, 0:64] = 1.0
        selm[2 * pr + 1, pr, 64:128] = 1.0
    w['sel'] = selm.astype(BF)
    return w


def get_compiled():
    if 'nc' in _STATE:
        return _STATE['nc'], _STATE['cfg']
    import concourse.bacc as bacc
    cfg = Cfg()
    nc = bacc.Bacc("TRN2", target_bir_lowering=False, debug=False,
                   num_devices=8)
    build(nc, cfg)
    nc.compile()
    _STATE['nc'], _STATE['cfg'] = nc, cfg
    return nc, cfg


def kernel(**inputs):
    from concourse import bass_utils
    x = np.ascontiguousarray(np.asarray(inputs['x'], dtype=np.float32))
    B, T, C = x.shape
    nc, cfg = get_compiled()
    w = prep_weights(inputs, cfg)
    in_maps = []
    perms = []
    for core in range(8):
        b, pid = core // 2, core % 2
        perm = core_perm(pid, T)
        perms.append((b, perm))
        m, mbt = build_masks_np(perm, T)
        im = dict(w)
        xc = np.ascontiguousarray(x[b][perm])
        im['xp'] = xc
        im['xpb'] = xc.astype(BF)
        im['masks'] = m
        im['mbias'] = mbt
        in_maps.append(im)
    res = bass_utils.run_bass_kernel_spmd(nc, in_maps,
                                          core_ids=list(range(8)),
                                          **_STATE.get('run_kwargs', {}))
    y = np.zeros((B, T, C), np.float32)
    for core in range(8):
        b, perm = perms[core]
        y[b][perm[:T // 2]] = res.results[core]['y'].astype(np.float32)
    _STATE['last_result'] = res
    return y
